# revision 7
# baseline (speedup 1.0000x reference)
"""Trainium2 Bass kernel for the GraphSearchPolicy forward pass.

Math (see reference):
    E = entity_emb[e]; Q = rel_emb[q]; X = [E, H, Q]
    X1 = relu(X @ W1 + b1); X2 = X1 @ W2 + b2
    logit[b,a] = <rel_emb[r_space[b,a]], X2[b]> - (1-r_mask)*HUGE
    dist = softmax(logit); entropy = -sum(dist*log(clip(dist,1e-20)))

Strategy: data-parallel over B across 8 NeuronCores (256 rows each).
Key trick: instead of gathering 33MB of rel_emb rows per core, compute
P[b, r] = <X2[b], rel_emb[r]> for ALL r in [0,1000) with one PE matmul
(X2T as stationary, rel_embT as moving), then gather the 256 needed
scores per row out of SBUF with gpsimd.ap_gather.

ap_gather shares one index list across each 16-partition group, so each
group's list is the concatenation (round-robin wrapped) of its 16 rows'
index lists; partition p's useful outputs land at free positions
16*t + (p%16). Those are extracted with a 0/1 mask multiply plus a
group-of-16 add-reduce on the DVE (junk lanes are zeroed exactly).
"""

import numpy as np

B, A = 2048, 256
NE, NR = 100000, 1000
DE, DR, DH = 128, 128, 256
ACTD = DE + DR            # 256, W1 output width
XDIM = DE + DH + DR       # 512
NCORES = 8
BC = B // NCORES          # 256 rows per core
NT = BC // 128            # 2 partition tiles per core
HUGE = float(np.float32(1e31))

_CACHE = {}


def _build(stage="full"):
    import concourse.bass as bass
    import concourse.tile as tile
    from concourse import bacc, mybir

    dt = mybir.dt
    Alu = mybir.AluOpType
    ActF = mybir.ActivationFunctionType

    nc = bacc.Bacc(
        "TRN2", target_bir_lowering=False, debug=False, num_devices=NCORES
    )

    d_ee = nc.dram_tensor("entity_emb", [NE, DE], dt.float32, kind="ExternalInput")
    d_re = nc.dram_tensor("rel_emb", [NR, DR], dt.float32, kind="ExternalInput")
    d_reT = nc.dram_tensor("rel_embT", [DR, NR], dt.float32, kind="ExternalInput")
    d_w1 = nc.dram_tensor("W1", [XDIM, ACTD], dt.float32, kind="ExternalInput")
    d_w2 = nc.dram_tensor("W2", [ACTD, DR], dt.float32, kind="ExternalInput")
    d_b1 = nc.dram_tensor("b1", [ACTD], dt.float32, kind="ExternalInput")
    d_b2 = nc.dram_tensor("b2", [DR], dt.float32, kind="ExternalInput")
    d_e = nc.dram_tensor("e_idx", [BC], dt.int32, kind="ExternalInput")
    d_q = nc.dram_tensor("q_idx", [BC], dt.int32, kind="ExternalInput")
    d_rs = nc.dram_tensor("rs16", [BC, A], dt.int16, kind="ExternalInput")
    d_rm = nc.dram_tensor("r_mask", [BC, A], dt.float32, kind="ExternalInput")
    d_h = nc.dram_tensor("H", [BC, DH], dt.float32, kind="ExternalInput")
    d_id = nc.dram_tensor("ident", [128, 128], dt.float32, kind="ExternalInput")
    d_mm = nc.dram_tensor("maskM", [128, 16], dt.float32, kind="ExternalInput")
    d_dist = nc.dram_tensor("dist", [BC, A], dt.float32, kind="ExternalOutput")
    d_dbg = None
    if stage == "p":
        d_dbg = nc.dram_tensor("dbgP", [BC, NR], dt.float32, kind="ExternalOutput")
    d_ent = nc.dram_tensor("entropy", [BC], dt.float32, kind="ExternalOutput")

    # partition-tiled DRAM views: row (t*128 + p) -> [p, t, ...]
    v_e = d_e.ap().rearrange("(t p) -> p t", p=128)
    v_q = d_q.ap().rearrange("(t p) -> p t", p=128)
    v_rs = d_rs.ap().rearrange("(t p) a -> p t a", p=128)
    v_rm = d_rm.ap().rearrange("(t p) a -> p t a", p=128)
    v_h = d_h.ap().rearrange("(t p) a -> p t a", p=128)
    v_w1 = d_w1.ap().rearrange("(t k) m -> k t m", k=128)   # [128, 4, 256]
    v_w2 = d_w2.ap().rearrange("(t k) m -> k t m", k=128)   # [128, 2, 128]
    v_b1 = d_b1.ap().rearrange("(t p) -> p t", p=128)       # [128, 2]
    v_b2 = d_b2.ap().rearrange("(t p) -> p t", p=128)       # [128, 1]
    v_reT = d_reT.ap().rearrange("(t p) r -> p t r", p=128)  # [128, 1, 1000]
    v_dist = d_dist.ap().rearrange("(t p) a -> p t a", p=128)
    v_ent = d_ent.ap().rearrange("(t p) -> p t", p=128)

    with tile.TileContext(nc) as tc:
        with (
            tc.tile_pool(name="const", bufs=1) as const,
            tc.tile_pool(name="work", bufs=2) as work,
            tc.tile_pool(name="big", bufs=2) as bigp,
            tc.tile_pool(name="psum", bufs=2, space="PSUM") as psum,
        ):
            # ---------------- input loads (spread across HWDGE queues) ---
            ident = const.tile([128, 128], dt.float32)
            nc.sync.dma_start(out=ident[:], in_=d_id.ap())
            maskM = const.tile([128, 16], dt.float32)
            nc.sync.dma_start(out=maskM[:], in_=d_mm.ap())
            w1s = const.tile([128, 4, ACTD], dt.float32)
            nc.sync.dma_start(out=w1s[:], in_=v_w1)
            w2s = const.tile([128, 2, DR], dt.float32)
            nc.sync.dma_start(out=w2s[:], in_=v_w2)

            b1s = const.tile([128, 2], dt.float32)
            nc.scalar.dma_start(out=b1s[:], in_=v_b1)
            b2s = const.tile([128, 1], dt.float32)
            nc.scalar.dma_start(out=b2s[:], in_=v_b2)
            e_sb = const.tile([128, NT], dt.int32)
            nc.scalar.dma_start(out=e_sb[:], in_=v_e)
            q_sb = const.tile([128, NT], dt.int32)
            nc.scalar.dma_start(out=q_sb[:], in_=v_q)
            rs_sb = const.tile([128, NT, A], dt.int16)
            nc.scalar.dma_start(out=rs_sb[:], in_=v_rs)

            rm_sb = const.tile([128, NT, A], dt.float32)
            nc.scalar.dma_start(out=rm_sb[:], in_=v_rm)
            h_sb = const.tile([128, NT, DH], dt.float32)
            nc.sync.dma_start(out=h_sb[:], in_=v_h)
            relT = const.tile([128, NR], dt.float32)
            nc.sync.dma_start(out=relT[:], in_=v_reT[:, 0, :])

            # ---------------- embedding gathers (Pool / SWDGE) ----------
            Eraw = work.tile([128, NT, DE], dt.float32, tag="Eraw")
            Qraw = work.tile([128, NT, DR], dt.float32, tag="Qraw")
            for t in range(NT):
                nc.gpsimd.indirect_dma_start(
                    out=Eraw[:, t, :],
                    out_offset=None,
                    in_=d_ee.ap(),
                    in_offset=bass.IndirectOffsetOnAxis(ap=e_sb[:, t : t + 1], axis=0),
                )
                nc.gpsimd.indirect_dma_start(
                    out=Qraw[:, t, :],
                    out_offset=None,
                    in_=d_re.ap(),
                    in_offset=bass.IndirectOffsetOnAxis(ap=q_sb[:, t : t + 1], axis=0),
                )

            # ---------------- build X^T k-tiles via PE transpose --------
            # XT[:, kt, :]: kt0 = E^T, kt1/kt2 = H^T halves, kt3 = Q^T
            XT = const.tile([128, 4, BC], dt.float32)
            for t in range(NT):
                bsl = slice(t * 128, t * 128 + 128)
                pE = psum.tile([128, 128], dt.float32, tag="ps_tr")
                nc.tensor.transpose(out=pE[:], in_=Eraw[:, t, :], identity=ident[:])
                nc.scalar.copy(out=XT[:, 0, bsl], in_=pE[:])
                for u in range(2):
                    pH = psum.tile([128, 128], dt.float32, tag="ps_tr")
                    nc.tensor.transpose(
                        out=pH[:],
                        in_=h_sb[:, t, u * 128 : u * 128 + 128],
                        identity=ident[:],
                    )
                    nc.scalar.copy(out=XT[:, 1 + u, bsl], in_=pH[:])
                pQ = psum.tile([128, 128], dt.float32, tag="ps_tr")
                nc.tensor.transpose(out=pQ[:], in_=Qraw[:, t, :], identity=ident[:])
                nc.scalar.copy(out=XT[:, 3, bsl], in_=pQ[:])

            # ---------------- MLP: X1^T = relu(W1^T X^T + b1) -----------
            X1T = const.tile([128, 2, BC], dt.float32)
            for mt in range(2):
                ps = psum.tile([128, BC], dt.float32, tag="ps_mm")
                for kt in range(4):
                    nc.tensor.matmul(
                        ps[:],
                        w1s[:, kt, mt * 128 : mt * 128 + 128],
                        XT[:, kt, :],
                        start=(kt == 0),
                        stop=(kt == 3),
                    )
                nc.scalar.activation(
                    out=X1T[:, mt, :],
                    in_=ps[:],
                    func=ActF.Relu,
                    bias=b1s[:, mt : mt + 1],
                    scale=1.0,
                )

            # ---------------- X2^T = W2^T X1^T + b2 ---------------------
            X2T = const.tile([128, BC], dt.float32)
            ps2 = psum.tile([128, BC], dt.float32, tag="ps_mm")
            for kt in range(2):
                nc.tensor.matmul(
                    ps2[:],
                    w2s[:, kt, :],
                    X1T[:, kt, :],
                    start=(kt == 0),
                    stop=(kt == 1),
                )
            nc.scalar.activation(
                out=X2T[:], in_=ps2[:], func=ActF.Identity,
                bias=b2s[:, 0:1], scale=1.0,
            )

            # ---------------- P[b, r] = X2 @ rel_emb^T ------------------
            P_sb = const.tile([128, NT, NR], dt.float32)
            for t in range(NT):
                for n0 in (0, 500):
                    pp = psum.tile([128, 500], dt.float32, tag="ps_p")
                    nc.tensor.matmul(
                        pp[:],
                        X2T[:, t * 128 : t * 128 + 128],
                        relT[:, n0 : n0 + 500],
                        start=True,
                        stop=True,
                    )
                    nc.scalar.copy(out=P_sb[:, t, n0 : n0 + 500], in_=pp[:])

            if stage == "p":
                v_dbg = d_dbg.ap().rearrange("(t p) r -> p t r", p=128)
                for t in range(NT):
                    nc.sync.dma_start(out=v_dbg[:, t, :], in_=P_sb[:, t, :])
                # dummy writes to other outputs so they're bound
                z = work.tile([128, NT, A], dt.float32, tag="zz")
                nc.vector.memset(z[:], 0.0)
                for t in range(NT):
                    nc.sync.dma_start(out=v_dist[:, t, :], in_=z[:, t, :])
                    nc.sync.dma_start(out=v_ent[:, t : t + 1], in_=z[:, t, 0:1])
            # ---------------- gather + extract + softmax ----------------
            mask_bc = bass.AP(
                tensor=maskM[:].tensor,
                offset=maskM[:].offset,
                ap=[maskM[:].ap[0], [0, A], maskM[:].ap[1]],
            )
            for t in range(NT if stage != "p" else 0):
                big = bigp.tile([128, A * 16], dt.float32, tag="big")
                nc.gpsimd.ap_gather(
                    big[:],
                    P_sb[:, t, :],
                    rs_sb[:, t, :],
                    channels=128,
                    num_elems=NR,
                    d=1,
                    num_idxs=A * 16,
                )
                # zero the 15/16 junk lanes, then add-reduce groups of 16
                nc.vector.tensor_tensor(
                    out=big[:], in0=big[:], in1=mask_bc, op=Alu.mult
                )
                l_t = work.tile([128, A], dt.float32, tag="l")
                nc.vector.tensor_reduce(
                    out=l_t[:],
                    in_=big[:].rearrange("p (s j) -> p s j", j=16),
                    axis=mybir.AxisListType.X,
                    op=Alu.add,
                )
                if stage == "gather":
                    nc.sync.dma_start(out=v_dist[:, t, :], in_=l_t[:])
                    nc.sync.dma_start(out=v_ent[:, t : t + 1], in_=l_t[:, 0:1])
                    continue
                # logit mask: l += r_mask*HUGE - HUGE  (== -(1-r_mask)*HUGE)
                mterm = work.tile([128, A], dt.float32, tag="mterm")
                nc.vector.tensor_scalar(
                    out=mterm[:], in0=rm_sb[:, t, :],
                    scalar1=HUGE, scalar2=-HUGE,
                    op0=Alu.mult, op1=Alu.add,
                )
                lm = work.tile([128, A], dt.float32, tag="lm")
                nc.vector.tensor_tensor(
                    out=lm[:], in0=l_t[:], in1=mterm[:], op=Alu.add
                )
                # softmax
                negm = work.tile([128, 1], dt.float32, tag="negm")
                nc.vector.tensor_reduce(
                    out=negm[:], in_=lm[:], axis=mybir.AxisListType.X,
                    op=Alu.max, negate=True,
                )
                et = work.tile([128, A], dt.float32, tag="et")
                zs = work.tile([128, 1], dt.float32, tag="zs")
                nc.scalar.activation(
                    out=et[:], in_=lm[:], func=ActF.Exp,
                    bias=negm[:], scale=1.0, accum_out=zs[:],
                )
                rz = work.tile([128, 1], dt.float32, tag="rz")
                nc.vector.reciprocal(out=rz[:], in_=zs[:])
                dist_t = work.tile([128, A], dt.float32, tag="dist")
                nc.vector.tensor_scalar(
                    out=dist_t[:], in0=et[:],
                    scalar1=rz[:], scalar2=None, op0=Alu.mult,
                )
                if stage == "dist":
                    nc.sync.dma_start(out=v_dist[:, t, :], in_=dist_t[:])
                    nc.sync.dma_start(out=v_ent[:, t : t + 1], in_=dist_t[:, 0:1])
                    continue
                # entropy = -sum(dist * log(max(dist, 1e-20)))
                cl = work.tile([128, A], dt.float32, tag="cl")
                nc.vector.tensor_scalar(
                    out=cl[:], in0=dist_t[:],
                    scalar1=1e-20, scalar2=None, op0=Alu.max,
                )
                lg = work.tile([128, A], dt.float32, tag="lg")
                nc.scalar.activation(out=lg[:], in_=cl[:], func=ActF.Ln)
                scr = work.tile([128, A], dt.float32, tag="scr")
                ent = work.tile([128, 1], dt.float32, tag="ent")
                nc.vector.tensor_tensor(
                    out=scr[:], in0=dist_t[:], in1=lg[:], op=Alu.mult
                )
                nc.vector.tensor_reduce(
                    out=ent[:], in_=scr[:], axis=mybir.AxisListType.X,
                    op=Alu.add, negate=True,
                )
                nc.sync.dma_start(out=v_dist[:, t, :], in_=dist_t[:])
                nc.sync.dma_start(out=v_ent[:, t : t + 1], in_=ent[:])

    nc.compile()
    return nc


def _get_nc():
    if "nc" not in _CACHE:
        _CACHE["nc"] = _build()
    return _CACHE["nc"]


def _ensure_ntff_hook():
    """Install the antenv.axon_hooks shim so trace=True works under axon."""
    import sys
    import types

    try:
        from antenv.axon_hooks import get_axon_ntff_profile_hook  # noqa: F401
        return
    except ImportError:
        pass
    try:
        from trn_agent_boot.trn_boot import _ntff_profile_via_ctypes
        hook = _ntff_profile_via_ctypes("/opt/axon/libaxon_pjrt.so")
    except Exception:
        hook = None
    mod = types.ModuleType("antenv.axon_hooks")
    mod.get_axon_ntff_profile_hook = lambda: hook
    mod.set_axon_ntff_profile_hook = lambda h: None
    sys.modules["antenv.axon_hooks"] = mod
    import antenv

    antenv.axon_hooks = mod


def kernel(e, q, H, r_space, r_mask, entity_emb, rel_emb, W1, b1, W2, b2,
           trace=False, **run_kwargs):
    from concourse.bass_utils import run_bass_kernel_spmd

    if trace:
        _ensure_ntff_hook()

    e = np.asarray(e).astype(np.int32)
    q = np.asarray(q).astype(np.int32)
    H = np.ascontiguousarray(np.asarray(H, dtype=np.float32))
    r_space16 = np.asarray(r_space).astype(np.int16)
    r_mask = np.ascontiguousarray(np.asarray(r_mask, dtype=np.float32))
    entity_emb = np.ascontiguousarray(np.asarray(entity_emb, dtype=np.float32))
    rel_emb = np.ascontiguousarray(np.asarray(rel_emb, dtype=np.float32))
    rel_embT = np.ascontiguousarray(rel_emb.T)
    W1 = np.ascontiguousarray(np.asarray(W1, dtype=np.float32))
    W2 = np.ascontiguousarray(np.asarray(W2, dtype=np.float32))
    b1 = np.ascontiguousarray(np.asarray(b1, dtype=np.float32))
    b2 = np.ascontiguousarray(np.asarray(b2, dtype=np.float32))
    ident = np.eye(128, dtype=np.float32)
    maskM = np.zeros((128, 16), dtype=np.float32)
    maskM[np.arange(128), np.arange(128) % 16] = 1.0

    nc = _get_nc()
    in_maps = []
    for i in range(NCORES):
        sl = slice(i * BC, (i + 1) * BC)
        in_maps.append({
            "entity_emb": entity_emb,
            "rel_emb": rel_emb,
            "rel_embT": rel_embT,
            "W1": W1, "W2": W2, "b1": b1, "b2": b2,
            "e_idx": np.ascontiguousarray(e[sl]),
            "q_idx": np.ascontiguousarray(q[sl]),
            "rs16": np.ascontiguousarray(r_space16[sl]),
            "r_mask": np.ascontiguousarray(r_mask[sl]),
            "H": np.ascontiguousarray(H[sl]),
            "ident": ident,
            "maskM": maskM,
        })

    res = run_bass_kernel_spmd(
        nc, in_maps, core_ids=list(range(NCORES)), trace=trace, **run_kwargs
    )
    _CACHE["last_result"] = res

    dist = np.concatenate([r["dist"] for r in res.results], axis=0)
    entropy = np.concatenate([r["entropy"] for r in res.results], axis=0)
    return dist, entropy


# revision 8
# speedup vs baseline: 3.5975x; 3.5975x over previous
"""Trainium2 Bass kernel for the GraphSearchPolicy forward pass.

Math (see reference):
    E = entity_emb[e]; Q = rel_emb[q]; X = [E, H, Q]
    X1 = relu(X @ W1 + b1); X2 = X1 @ W2 + b2
    logit[b,a] = <rel_emb[r_space[b,a]], X2[b]> - (1-r_mask)*HUGE
    dist = softmax(logit); entropy = -sum(dist*log(clip(dist,1e-20)))

Strategy: data-parallel over B across 8 NeuronCores (256 rows each).

Key trick 1: instead of gathering 33MB of rel_emb rows per core, compute
P[b, r] = <X2[b], rel_emb[r]> for ALL r in [0,1000) with PE matmuls
(X2T stationary, rel_embT moving).

Key trick 2: the per-row gather logit[b,a] = P[b, r_space[b,a]] is
executed as a per-partition INVERSE SCATTER with gpsimd.local_scatter
(a fast streaming Q7 kernel with true per-partition indices):
  round 0:  dst[p, inv0[p,r]] = P[p,r]   (first occurrence of each r)
  round k:  dst[p, jump_{2^(k-1)}[p,a]] = cur[p,a], cur |= dst
where jump_j maps occurrence i of a value to occurrence i+j. After
ceil(log2(max_multiplicity)) rounds every duplicate slot is filled.
All maps are integer-only functions of r_space, precomputed host-side.
f32 values are moved bit-exactly as u16 pairs (local_scatter is 16-bit).
"""

import numpy as np

B, A = 2048, 256
NE, NR = 100000, 1000
DE, DR, DH = 128, 128, 256
ACTD = DE + DR            # 256, W1 output width
XDIM = DE + DH + DR       # 512
NCORES = 8
BC = B // NCORES          # 256 rows per core
NT = BC // 128            # 2 partition tiles per core
HUGE = float(np.float32(1e31))

_CACHE = {}


def _build_maps(r_space):
    """Integer-only index prep: inverse first-occurrence map plus
    log-doubling duplicate jump maps, all derived from r_space."""
    s = np.asarray(r_space).astype(np.int64)
    Bn, An = s.shape
    order = np.argsort(s, axis=1, kind="stable")
    s_sorted = np.take_along_axis(s, order, axis=1)
    new_grp = np.ones_like(s_sorted, dtype=bool)
    new_grp[:, 1:] = s_sorted[:, 1:] != s_sorted[:, :-1]
    idx_all = np.broadcast_to(np.arange(An)[None, :], (Bn, An))
    grp_start = np.where(new_grp, idx_all, 0)
    grp_start = np.maximum.accumulate(grp_start, axis=1)
    occ_sorted = idx_all - grp_start
    occ = np.empty_like(occ_sorted)
    np.put_along_axis(occ, order, occ_sorted, axis=1)
    M = int(occ.max()) + 1

    inv0 = np.full((Bn, NR), -1, np.int64)
    rows, cols = np.nonzero(occ == 0)
    inv0[rows, s[rows, cols]] = cols

    R = 0 if M <= 1 else int(np.ceil(np.log2(M)))
    jumps = []
    j = 1
    for _ in range(R):
        nxt_sorted = np.full((Bn, An), -1, np.int64)
        same = s_sorted[:, j:] == s_sorted[:, :-j]
        tmp = nxt_sorted[:, :-j]
        tmp[same] = order[:, j:][same]
        nxt_sorted[:, :-j] = tmp
        nxt = np.empty_like(nxt_sorted)
        np.put_along_axis(nxt, order, nxt_sorted, axis=1)
        jumps.append(nxt)
        j *= 2
    return inv0, jumps, M, R


def _pairs16(m):
    """Expand an f32-slot index map to its u16-pair index map (-1 kept)."""
    out = np.full((m.shape[0], 2 * m.shape[1]), -1, np.int16)
    valid = m >= 0
    out[:, 0::2] = np.where(valid, 2 * m, -1).astype(np.int16)
    out[:, 1::2] = np.where(valid, 2 * m + 1, -1).astype(np.int16)
    return out


def _build(rounds):
    import concourse.bass as bass
    import concourse.tile as tile
    from concourse import bacc, mybir

    dt = mybir.dt
    Alu = mybir.AluOpType
    ActF = mybir.ActivationFunctionType

    nc = bacc.Bacc(
        "TRN2", target_bir_lowering=False, debug=False, num_devices=NCORES
    )

    d_ee = nc.dram_tensor("entity_emb", [NE, DE], dt.float32, kind="ExternalInput")
    d_re = nc.dram_tensor("rel_emb", [NR, DR], dt.float32, kind="ExternalInput")
    d_reT = nc.dram_tensor("rel_embT", [DR, NR], dt.float32, kind="ExternalInput")
    d_w1 = nc.dram_tensor("W1", [XDIM, ACTD], dt.float32, kind="ExternalInput")
    d_w2 = nc.dram_tensor("W2", [ACTD, DR], dt.float32, kind="ExternalInput")
    d_b1 = nc.dram_tensor("b1", [ACTD], dt.float32, kind="ExternalInput")
    d_b2 = nc.dram_tensor("b2", [DR], dt.float32, kind="ExternalInput")
    d_e = nc.dram_tensor("e_idx", [BC], dt.int32, kind="ExternalInput")
    d_q = nc.dram_tensor("q_idx", [BC], dt.int32, kind="ExternalInput")
    d_inv = nc.dram_tensor("inv16", [BC, 2 * NR], dt.int16, kind="ExternalInput")
    d_jmp = [
        nc.dram_tensor(f"jmp{k}", [BC, 2 * A], dt.int16, kind="ExternalInput")
        for k in range(rounds)
    ]
    d_rm = nc.dram_tensor("r_mask", [BC, A], dt.float32, kind="ExternalInput")
    d_h = nc.dram_tensor("H", [BC, DH], dt.float32, kind="ExternalInput")
    d_id = nc.dram_tensor("ident", [128, 128], dt.float32, kind="ExternalInput")
    d_dist = nc.dram_tensor("dist", [BC, A], dt.float32, kind="ExternalOutput")
    d_ent = nc.dram_tensor("entropy", [BC], dt.float32, kind="ExternalOutput")

    # partition-tiled DRAM views: row (t*128 + p) -> [p, t, ...]
    v_e = d_e.ap().rearrange("(t p) -> p t", p=128)
    v_q = d_q.ap().rearrange("(t p) -> p t", p=128)
    v_inv = d_inv.ap().rearrange("(t p) c -> p t c", p=128)
    v_jmp = [d.ap().rearrange("(t p) c -> p t c", p=128) for d in d_jmp]
    v_rm = d_rm.ap().rearrange("(t p) a -> p t a", p=128)
    v_h = d_h.ap().rearrange("(t p) a -> p t a", p=128)
    v_w1 = d_w1.ap().rearrange("(t k) m -> k t m", k=128)   # [128, 4, 256]
    v_w2 = d_w2.ap().rearrange("(t k) m -> k t m", k=128)   # [128, 2, 128]
    v_b1 = d_b1.ap().rearrange("(t p) -> p t", p=128)       # [128, 2]
    v_b2 = d_b2.ap().rearrange("(t p) -> p t", p=128)       # [128, 1]
    v_reT = d_reT.ap().rearrange("(t p) r -> p t r", p=128)  # [128, 1, 1000]
    v_dist = d_dist.ap().rearrange("(t p) a -> p t a", p=128)
    v_ent = d_ent.ap().rearrange("(t p) -> p t", p=128)

    with tile.TileContext(nc) as tc:
        with (
            tc.tile_pool(name="const", bufs=1) as const,
            tc.tile_pool(name="work", bufs=2) as work,
            tc.tile_pool(name="psum", bufs=2, space="PSUM") as psum,
        ):
            # ---------------- input loads (spread across HWDGE queues) ---
            ident = const.tile([128, 128], dt.float32)
            nc.sync.dma_start(out=ident[:], in_=d_id.ap())
            w1s = const.tile([128, 4, ACTD], dt.float32)
            nc.sync.dma_start(out=w1s[:], in_=v_w1)
            w2s = const.tile([128, 2, DR], dt.float32)
            nc.sync.dma_start(out=w2s[:], in_=v_w2)
            h_sb = const.tile([128, NT, DH], dt.float32)
            nc.sync.dma_start(out=h_sb[:], in_=v_h)
            relT = const.tile([128, NR], dt.float32)
            nc.sync.dma_start(out=relT[:], in_=v_reT[:, 0, :])

            b1s = const.tile([128, 2], dt.float32)
            nc.scalar.dma_start(out=b1s[:], in_=v_b1)
            b2s = const.tile([128, 1], dt.float32)
            nc.scalar.dma_start(out=b2s[:], in_=v_b2)
            e_sb = const.tile([128, NT], dt.int32)
            nc.scalar.dma_start(out=e_sb[:], in_=v_e)
            q_sb = const.tile([128, NT], dt.int32)
            nc.scalar.dma_start(out=q_sb[:], in_=v_q)
            inv_sb = const.tile([128, NT, 2 * NR], dt.int16)
            nc.scalar.dma_start(out=inv_sb[:], in_=v_inv)
            jmp_sb = []
            for k in range(rounds):
                jt = const.tile([128, NT, 2 * A], dt.int16, tag=f"jmp{k}")
                nc.scalar.dma_start(out=jt[:], in_=v_jmp[k])
                jmp_sb.append(jt)
            rm_sb = const.tile([128, NT, A], dt.float32)
            nc.scalar.dma_start(out=rm_sb[:], in_=v_rm)

            # ---------------- embedding gathers (Pool / SWDGE) ----------
            Eraw = work.tile([128, NT, DE], dt.float32, tag="Eraw")
            Qraw = work.tile([128, NT, DR], dt.float32, tag="Qraw")
            for t in range(NT):
                nc.gpsimd.indirect_dma_start(
                    out=Eraw[:, t, :],
                    out_offset=None,
                    in_=d_ee.ap(),
                    in_offset=bass.IndirectOffsetOnAxis(ap=e_sb[:, t : t + 1], axis=0),
                )
                nc.gpsimd.indirect_dma_start(
                    out=Qraw[:, t, :],
                    out_offset=None,
                    in_=d_re.ap(),
                    in_offset=bass.IndirectOffsetOnAxis(ap=q_sb[:, t : t + 1], axis=0),
                )

            # ---------------- build X^T k-tiles via PE transpose --------
            # XT[:, kt, :]: kt0 = E^T, kt1/kt2 = H^T halves, kt3 = Q^T
            XT = const.tile([128, 4, BC], dt.float32)
            for t in range(NT):
                bsl = slice(t * 128, t * 128 + 128)
                pE = psum.tile([128, 128], dt.float32, tag="ps_tr")
                nc.tensor.transpose(out=pE[:], in_=Eraw[:, t, :], identity=ident[:])
                nc.scalar.copy(out=XT[:, 0, bsl], in_=pE[:])
                for u in range(2):
                    pH = psum.tile([128, 128], dt.float32, tag="ps_tr")
                    nc.tensor.transpose(
                        out=pH[:],
                        in_=h_sb[:, t, u * 128 : u * 128 + 128],
                        identity=ident[:],
                    )
                    nc.scalar.copy(out=XT[:, 1 + u, bsl], in_=pH[:])
                pQ = psum.tile([128, 128], dt.float32, tag="ps_tr")
                nc.tensor.transpose(out=pQ[:], in_=Qraw[:, t, :], identity=ident[:])
                nc.scalar.copy(out=XT[:, 3, bsl], in_=pQ[:])

            # ---------------- MLP: X1^T = relu(W1^T X^T + b1) -----------
            X1T = const.tile([128, 2, BC], dt.float32)
            for mt in range(2):
                ps = psum.tile([128, BC], dt.float32, tag="ps_mm")
                for kt in range(4):
                    nc.tensor.matmul(
                        ps[:],
                        w1s[:, kt, mt * 128 : mt * 128 + 128],
                        XT[:, kt, :],
                        start=(kt == 0),
                        stop=(kt == 3),
                    )
                nc.scalar.activation(
                    out=X1T[:, mt, :],
                    in_=ps[:],
                    func=ActF.Relu,
                    bias=b1s[:, mt : mt + 1],
                    scale=1.0,
                )

            # ---------------- X2^T = W2^T X1^T + b2 ---------------------
            X2T = const.tile([128, BC], dt.float32)
            ps2 = psum.tile([128, BC], dt.float32, tag="ps_mm")
            for kt in range(2):
                nc.tensor.matmul(
                    ps2[:],
                    w2s[:, kt, :],
                    X1T[:, kt, :],
                    start=(kt == 0),
                    stop=(kt == 1),
                )
            nc.scalar.activation(
                out=X2T[:], in_=ps2[:], func=ActF.Identity,
                bias=b2s[:, 0:1], scale=1.0,
            )

            # ---------------- P[b, r] = X2 @ rel_emb^T ------------------
            # stored as int16 so local_scatter can stream it; f32 written
            # through a bitcast view.
            P16 = const.tile([128, NT, 2 * NR], dt.int16)
            for t in range(NT):
                for n0 in (0, 500):
                    pp = psum.tile([128, 500], dt.float32, tag="ps_p")
                    nc.tensor.matmul(
                        pp[:],
                        X2T[:, t * 128 : t * 128 + 128],
                        relT[:, n0 : n0 + 500],
                        start=True,
                        stop=True,
                    )
                    nc.scalar.copy(
                        out=P16[:, t, 2 * n0 : 2 * n0 + 1000].bitcast(dt.float32),
                        in_=pp[:],
                    )

            # ------------- scatter rounds + softmax per tile ------------
            for t in range(NT):
                lt16 = work.tile([128, 2 * A], dt.int16, tag="lt16")
                nc.gpsimd.local_scatter(
                    lt16[:], P16[:, t, :], inv_sb[:, t, :],
                    channels=128, num_elems=2 * A, num_idxs=2 * NR,
                )
                for k in range(rounds):
                    dk = work.tile([128, 2 * A], dt.int16, tag="dk")
                    nc.gpsimd.local_scatter(
                        dk[:], lt16[:], jmp_sb[k][:, t, :],
                        channels=128, num_elems=2 * A, num_idxs=2 * A,
                    )
                    nc.vector.tensor_tensor(
                        out=lt16[:], in0=lt16[:], in1=dk[:], op=Alu.bitwise_or
                    )
                l_t = lt16[:].bitcast(dt.float32)   # [128, A]
                # logit mask: l += r_mask*HUGE - HUGE  (== -(1-r_mask)*HUGE)
                mterm = work.tile([128, A], dt.float32, tag="mterm")
                nc.vector.tensor_scalar(
                    out=mterm[:], in0=rm_sb[:, t, :],
                    scalar1=HUGE, scalar2=-HUGE,
                    op0=Alu.mult, op1=Alu.add,
                )
                lm = work.tile([128, A], dt.float32, tag="lm")
                nc.vector.tensor_tensor(
                    out=lm[:], in0=l_t, in1=mterm[:], op=Alu.add
                )
                # softmax
                negm = work.tile([128, 1], dt.float32, tag="negm")
                nc.vector.tensor_reduce(
                    out=negm[:], in_=lm[:], axis=mybir.AxisListType.X,
                    op=Alu.max, negate=True,
                )
                et = work.tile([128, A], dt.float32, tag="et")
                zs = work.tile([128, 1], dt.float32, tag="zs")
                nc.scalar.activation(
                    out=et[:], in_=lm[:], func=ActF.Exp,
                    bias=negm[:], scale=1.0, accum_out=zs[:],
                )
                rz = work.tile([128, 1], dt.float32, tag="rz")
                nc.vector.reciprocal(out=rz[:], in_=zs[:])
                dist_t = work.tile([128, A], dt.float32, tag="dist")
                nc.vector.tensor_scalar(
                    out=dist_t[:], in0=et[:],
                    scalar1=rz[:], scalar2=None, op0=Alu.mult,
                )
                # entropy = -sum(dist * log(max(dist, 1e-20)))
                cl = work.tile([128, A], dt.float32, tag="cl")
                nc.vector.tensor_scalar(
                    out=cl[:], in0=dist_t[:],
                    scalar1=1e-20, scalar2=None, op0=Alu.max,
                )
                lg = work.tile([128, A], dt.float32, tag="lg")
                nc.scalar.activation(out=lg[:], in_=cl[:], func=ActF.Ln)
                scr = work.tile([128, A], dt.float32, tag="scr")
                ent = work.tile([128, 1], dt.float32, tag="ent")
                nc.vector.tensor_tensor(
                    out=scr[:], in0=dist_t[:], in1=lg[:], op=Alu.mult
                )
                nc.vector.tensor_reduce(
                    out=ent[:], in_=scr[:], axis=mybir.AxisListType.X,
                    op=Alu.add, negate=True,
                )
                nc.sync.dma_start(out=v_dist[:, t, :], in_=dist_t[:])
                nc.sync.dma_start(out=v_ent[:, t : t + 1], in_=ent[:])

    nc.compile()
    return nc


def _get_nc(rounds):
    key = ("nc", rounds)
    if key not in _CACHE:
        _CACHE[key] = _build(rounds)
    return _CACHE[key]


def _ensure_ntff_hook():
    """Install the antenv.axon_hooks shim so trace=True works under axon."""
    import sys
    import types

    try:
        from antenv.axon_hooks import get_axon_ntff_profile_hook  # noqa: F401
        return
    except ImportError:
        pass
    try:
        from trn_agent_boot.trn_boot import _ntff_profile_via_ctypes
        hook = _ntff_profile_via_ctypes("/opt/axon/libaxon_pjrt.so")
    except Exception:
        hook = None
    mod = types.ModuleType("antenv.axon_hooks")
    mod.get_axon_ntff_profile_hook = lambda: hook
    mod.set_axon_ntff_profile_hook = lambda h: None
    sys.modules["antenv.axon_hooks"] = mod
    import antenv

    antenv.axon_hooks = mod


def kernel(e, q, H, r_space, r_mask, entity_emb, rel_emb, W1, b1, W2, b2,
           trace=False, **run_kwargs):
    from concourse.bass_utils import run_bass_kernel_spmd

    if trace:
        _ensure_ntff_hook()

    e = np.asarray(e).astype(np.int32)
    q = np.asarray(q).astype(np.int32)
    H = np.ascontiguousarray(np.asarray(H, dtype=np.float32))
    r_space = np.asarray(r_space)
    r_mask = np.ascontiguousarray(np.asarray(r_mask, dtype=np.float32))
    entity_emb = np.ascontiguousarray(np.asarray(entity_emb, dtype=np.float32))
    rel_emb = np.ascontiguousarray(np.asarray(rel_emb, dtype=np.float32))
    rel_embT = np.ascontiguousarray(rel_emb.T)
    W1 = np.ascontiguousarray(np.asarray(W1, dtype=np.float32))
    W2 = np.ascontiguousarray(np.asarray(W2, dtype=np.float32))
    b1 = np.ascontiguousarray(np.asarray(b1, dtype=np.float32))
    b2 = np.ascontiguousarray(np.asarray(b2, dtype=np.float32))
    ident = np.eye(128, dtype=np.float32)

    inv0, jumps, M, R = _build_maps(r_space)
    inv16 = _pairs16(inv0)
    jmp16 = [_pairs16(j) for j in jumps]

    nc = _get_nc(R)
    in_maps = []
    for i in range(NCORES):
        sl = slice(i * BC, (i + 1) * BC)
        m = {
            "entity_emb": entity_emb,
            "rel_emb": rel_emb,
            "rel_embT": rel_embT,
            "W1": W1, "W2": W2, "b1": b1, "b2": b2,
            "e_idx": np.ascontiguousarray(e[sl]),
            "q_idx": np.ascontiguousarray(q[sl]),
            "inv16": np.ascontiguousarray(inv16[sl]),
            "r_mask": np.ascontiguousarray(r_mask[sl]),
            "H": np.ascontiguousarray(H[sl]),
            "ident": ident,
        }
        for k in range(R):
            m[f"jmp{k}"] = np.ascontiguousarray(jmp16[k][sl])
        in_maps.append(m)

    res = run_bass_kernel_spmd(
        nc, in_maps, core_ids=list(range(NCORES)), trace=trace, **run_kwargs
    )
    _CACHE["last_result"] = res

    dist = np.concatenate([r["dist"] for r in res.results], axis=0)
    entropy = np.concatenate([r["entropy"] for r in res.results], axis=0)
    return dist, entropy


# revision 10
# speedup vs baseline: 3.8538x; 1.0713x over previous
"""Trainium2 Bass kernel for the GraphSearchPolicy forward pass.

Math (see reference):
    E = entity_emb[e]; Q = rel_emb[q]; X = [E, H, Q]
    X1 = relu(X @ W1 + b1); X2 = X1 @ W2 + b2
    logit[b,a] = <rel_emb[r_space[b,a]], X2[b]> - (1-r_mask)*HUGE
    dist = softmax(logit); entropy = -sum(dist*log(clip(dist,1e-20)))

Strategy: data-parallel over B across 8 NeuronCores (256 rows each).

Key trick 1: instead of gathering 33MB of rel_emb rows per core, compute
P[b, r] = <X2[b], rel_emb[r]> for ALL r in [0,1000) with PE matmuls
(X2T stationary, rel_embT moving).

Key trick 2: the per-row gather logit[b,a] = P[b, r_space[b,a]] is
executed as a per-partition INVERSE SCATTER with gpsimd.local_scatter
(a fast streaming Q7 kernel with true per-partition indices):
  round 0:  dst[p, inv0[p,r]] = P[p,r]   (first occurrence of each r)
  round k:  dst[p, jump_{2^(k-1)}[p,a]] = cur[p,a], cur |= dst
where jump_j maps occurrence i of a value to occurrence i+j. After
ceil(log2(max_multiplicity)) rounds every duplicate slot is filled.
All maps are integer-only functions of r_space, precomputed host-side.
f32 values are moved bit-exactly as u16 pairs (local_scatter is 16-bit).
"""

import numpy as np

B, A = 2048, 256
NE, NR = 100000, 1000
DE, DR, DH = 128, 128, 256
ACTD = DE + DR            # 256, W1 output width
XDIM = DE + DH + DR       # 512
NCORES = 8
BC = B // NCORES          # 256 rows per core
NT = BC // 128            # 2 partition tiles per core
HUGE = float(np.float32(1e31))

_CACHE = {}


def _build_maps(r_space):
    """Integer-only index prep: inverse first-occurrence map plus
    log-doubling duplicate jump maps, all derived from r_space."""
    s = np.asarray(r_space).astype(np.int64)
    Bn, An = s.shape
    order = np.argsort(s, axis=1, kind="stable")
    s_sorted = np.take_along_axis(s, order, axis=1)
    new_grp = np.ones_like(s_sorted, dtype=bool)
    new_grp[:, 1:] = s_sorted[:, 1:] != s_sorted[:, :-1]
    idx_all = np.broadcast_to(np.arange(An)[None, :], (Bn, An))
    grp_start = np.where(new_grp, idx_all, 0)
    grp_start = np.maximum.accumulate(grp_start, axis=1)
    occ_sorted = idx_all - grp_start
    occ = np.empty_like(occ_sorted)
    np.put_along_axis(occ, order, occ_sorted, axis=1)
    M = int(occ.max()) + 1

    inv0 = np.full((Bn, NR), -1, np.int64)
    rows, cols = np.nonzero(occ == 0)
    inv0[rows, s[rows, cols]] = cols

    R = 0 if M <= 1 else int(np.ceil(np.log2(M)))
    jumps = []
    j = 1
    for _ in range(R):
        nxt_sorted = np.full((Bn, An), -1, np.int64)
        same = s_sorted[:, j:] == s_sorted[:, :-j]
        tmp = nxt_sorted[:, :-j]
        tmp[same] = order[:, j:][same]
        nxt_sorted[:, :-j] = tmp
        nxt = np.empty_like(nxt_sorted)
        np.put_along_axis(nxt, order, nxt_sorted, axis=1)
        jumps.append(nxt)
        j *= 2
    return inv0, jumps, M, R


def _pairs16(m):
    """Expand an f32-slot index map to its u16-pair index map (-1 kept)."""
    out = np.full((m.shape[0], 2 * m.shape[1]), -1, np.int16)
    valid = m >= 0
    out[:, 0::2] = np.where(valid, 2 * m, -1).astype(np.int16)
    out[:, 1::2] = np.where(valid, 2 * m + 1, -1).astype(np.int16)
    return out


def _build(rounds):
    import concourse.bass as bass
    import concourse.tile as tile
    from concourse import bacc, mybir

    dt = mybir.dt
    Alu = mybir.AluOpType
    ActF = mybir.ActivationFunctionType

    nc = bacc.Bacc(
        "TRN2", target_bir_lowering=False, debug=False, num_devices=NCORES
    )

    d_ee = nc.dram_tensor("entity_emb", [NE, DE], dt.float32, kind="ExternalInput")
    d_re = nc.dram_tensor("rel_emb", [NR, DR], dt.float32, kind="ExternalInput")
    d_reT = nc.dram_tensor("rel_embT", [DR, NR], dt.float32, kind="ExternalInput")
    d_w1 = nc.dram_tensor("W1", [XDIM, ACTD], dt.float32, kind="ExternalInput")
    d_w2 = nc.dram_tensor("W2", [ACTD, DR], dt.float32, kind="ExternalInput")
    d_b1 = nc.dram_tensor("b1", [ACTD], dt.float32, kind="ExternalInput")
    d_b2 = nc.dram_tensor("b2", [DR], dt.float32, kind="ExternalInput")
    d_e = nc.dram_tensor("e_idx", [BC], dt.int32, kind="ExternalInput")
    d_q = nc.dram_tensor("q_idx", [BC], dt.int32, kind="ExternalInput")
    d_inv = nc.dram_tensor("inv16", [BC, 2 * NR], dt.int16, kind="ExternalInput")
    d_jmp = [
        nc.dram_tensor(f"jmp{k}", [BC, 2 * A], dt.int16, kind="ExternalInput")
        for k in range(rounds)
    ]
    d_rm = nc.dram_tensor("r_mask", [BC, A], dt.float32, kind="ExternalInput")
    d_h = nc.dram_tensor("H", [BC, DH], dt.float32, kind="ExternalInput")
    d_id = nc.dram_tensor("ident", [128, 128], dt.float32, kind="ExternalInput")
    d_dist = nc.dram_tensor("dist", [BC, A], dt.float32, kind="ExternalOutput")
    d_ent = nc.dram_tensor("entropy", [BC], dt.float32, kind="ExternalOutput")

    # partition-tiled DRAM views: row (t*128 + p) -> [p, t, ...]
    v_e = d_e.ap().rearrange("(t p) -> p t", p=128)
    v_q = d_q.ap().rearrange("(t p) -> p t", p=128)
    v_inv = d_inv.ap().rearrange("(t p) c -> p t c", p=128)
    v_jmp = [d.ap().rearrange("(t p) c -> p t c", p=128) for d in d_jmp]
    v_rm = d_rm.ap().rearrange("(t p) a -> p t a", p=128)
    v_h = d_h.ap().rearrange("(t p) a -> p t a", p=128)
    v_w1 = d_w1.ap().rearrange("(t k) m -> k t m", k=128)   # [128, 4, 256]
    v_w2 = d_w2.ap().rearrange("(t k) m -> k t m", k=128)   # [128, 2, 128]
    v_b1 = d_b1.ap().rearrange("(t p) -> p t", p=128)       # [128, 2]
    v_b2 = d_b2.ap().rearrange("(t p) -> p t", p=128)       # [128, 1]
    v_reT = d_reT.ap().rearrange("(t p) r -> p t r", p=128)  # [128, 1, 1000]
    v_dist = d_dist.ap().rearrange("(t p) a -> p t a", p=128)
    v_ent = d_ent.ap().rearrange("(t p) -> p t", p=128)

    with tile.TileContext(nc) as tc:
        with (
            tc.tile_pool(name="const", bufs=1) as const,
            tc.tile_pool(name="work", bufs=2) as work,
            tc.tile_pool(name="psum", bufs=2, space="PSUM") as psum,
        ):
            # ---------------- input loads (spread across HWDGE queues) ---
            ident = const.tile([128, 128], dt.float32)
            nc.sync.dma_start(out=ident[:], in_=d_id.ap())
            w1s = const.tile([128, 4, ACTD], dt.float32)
            nc.sync.dma_start(out=w1s[:], in_=v_w1)
            w2s = const.tile([128, 2, DR], dt.float32)
            nc.sync.dma_start(out=w2s[:], in_=v_w2)
            h_sb = const.tile([128, NT, DH], dt.float32)
            nc.sync.dma_start(out=h_sb[:], in_=v_h)
            relT = const.tile([128, NR], dt.float32)
            nc.sync.dma_start(out=relT[:], in_=v_reT[:, 0, :])

            e_sb = const.tile([128, NT], dt.int32)
            nc.scalar.dma_start(out=e_sb[:], in_=v_e)
            q_sb = const.tile([128, NT], dt.int32)
            nc.scalar.dma_start(out=q_sb[:], in_=v_q)
            b1s = const.tile([128, 2], dt.float32)
            nc.scalar.dma_start(out=b1s[:], in_=v_b1)
            b2s = const.tile([128, 1], dt.float32)
            nc.scalar.dma_start(out=b2s[:], in_=v_b2)
            rm_sb = const.tile([128, NT, A], dt.float32)
            nc.scalar.dma_start(out=rm_sb[:], in_=v_rm)
            inv_sb = const.tile([128, NT, 2 * NR], dt.int16)
            nc.scalar.dma_start(out=inv_sb[:], in_=v_inv)
            jmp_sb = []
            for k in range(rounds):
                jt = const.tile([128, NT, 2 * A], dt.int16, tag=f"jmp{k}")
                nc.sync.dma_start(out=jt[:], in_=v_jmp[k])
                jmp_sb.append(jt)

            # ---------------- embedding gathers (Pool / SWDGE) ----------
            Eraw = work.tile([128, NT, DE], dt.float32, tag="Eraw")
            Qraw = work.tile([128, NT, DR], dt.float32, tag="Qraw")
            for t in range(NT):
                nc.gpsimd.indirect_dma_start(
                    out=Eraw[:, t, :],
                    out_offset=None,
                    in_=d_ee.ap(),
                    in_offset=bass.IndirectOffsetOnAxis(ap=e_sb[:, t : t + 1], axis=0),
                )
                nc.gpsimd.indirect_dma_start(
                    out=Qraw[:, t, :],
                    out_offset=None,
                    in_=d_re.ap(),
                    in_offset=bass.IndirectOffsetOnAxis(ap=q_sb[:, t : t + 1], axis=0),
                )

            # ---------------- build X^T k-tiles via PE transpose --------
            # XT[:, kt, :]: kt0 = E^T, kt1/kt2 = H^T halves, kt3 = Q^T
            XT = const.tile([128, 4, BC], dt.float32)
            X1T = const.tile([128, 2, BC], dt.float32)
            X2T = const.tile([128, BC], dt.float32)
            P16 = const.tile([128, NT, 2 * NR], dt.int16)
            lt16 = const.tile([128, NT, 2 * A], dt.int16)
            ltf = lt16[:].rearrange("p t c -> p (t c)")          # [128, 1024]
            mterm = const.tile([128, NT, A], dt.float32)

            for t in range(NT):
                # mask term only needs r_mask; hoist off the tail
                nc.vector.tensor_scalar(
                    out=mterm[:, t, :], in0=rm_sb[:, t, :],
                    scalar1=HUGE, scalar2=-HUGE,
                    op0=Alu.mult, op1=Alu.add,
                )
                bsl = slice(t * 128, t * 128 + 128)
                pE = psum.tile([128, 128], dt.float32, tag="ps_tr")
                nc.tensor.transpose(out=pE[:], in_=Eraw[:, t, :], identity=ident[:])
                nc.scalar.copy(out=XT[:, 0, bsl], in_=pE[:])
                for u in range(2):
                    pH = psum.tile([128, 128], dt.float32, tag="ps_tr")
                    nc.tensor.transpose(
                        out=pH[:],
                        in_=h_sb[:, t, u * 128 : u * 128 + 128],
                        identity=ident[:],
                    )
                    nc.scalar.copy(out=XT[:, 1 + u, bsl], in_=pH[:])
                pQ = psum.tile([128, 128], dt.float32, tag="ps_tr")
                nc.tensor.transpose(out=pQ[:], in_=Qraw[:, t, :], identity=ident[:])
                nc.scalar.copy(out=XT[:, 3, bsl], in_=pQ[:])

                # ---- per-tile MLP so the first scatter starts early ----
                for mt in range(2):
                    ps = psum.tile([128, 128], dt.float32, tag="ps_mm")
                    for kt in range(4):
                        nc.tensor.matmul(
                            ps[:],
                            w1s[:, kt, mt * 128 : mt * 128 + 128],
                            XT[:, kt, bsl],
                            start=(kt == 0),
                            stop=(kt == 3),
                        )
                    nc.scalar.activation(
                        out=X1T[:, mt, bsl],
                        in_=ps[:],
                        func=ActF.Relu,
                        bias=b1s[:, mt : mt + 1],
                        scale=1.0,
                    )
                ps2 = psum.tile([128, 128], dt.float32, tag="ps_mm")
                for kt in range(2):
                    nc.tensor.matmul(
                        ps2[:],
                        w2s[:, kt, :],
                        X1T[:, kt, bsl],
                        start=(kt == 0),
                        stop=(kt == 1),
                    )
                nc.scalar.activation(
                    out=X2T[:, bsl], in_=ps2[:], func=ActF.Identity,
                    bias=b2s[:, 0:1], scale=1.0,
                )

                # ---- P[b, r] = X2 @ rel_emb^T, written as u16 pairs ----
                for n0 in (0, 500):
                    pp = psum.tile([128, 500], dt.float32, tag="ps_p")
                    nc.tensor.matmul(
                        pp[:],
                        X2T[:, bsl],
                        relT[:, n0 : n0 + 500],
                        start=True,
                        stop=True,
                    )
                    nc.scalar.copy(
                        out=P16[:, t, 2 * n0 : 2 * n0 + 1000].bitcast(dt.float32),
                        in_=pp[:],
                    )

                # ---- scatter round 0 (first occurrences) ----
                nc.gpsimd.local_scatter(
                    lt16[:, t, :], P16[:, t, :], inv_sb[:, t, :],
                    channels=128, num_elems=2 * A, num_idxs=2 * NR,
                )

            # ---- duplicate-fill jump rounds, both tiles per scatter ----
            for k in range(rounds):
                dk = work.tile([128, NT * 2 * A], dt.int16, tag="dk")
                nc.gpsimd.local_scatter(
                    dk[:], ltf, jmp_sb[k][:].rearrange("p t c -> p (t c)"),
                    channels=128, num_elems=NT * 2 * A, num_idxs=NT * 2 * A,
                )
                nc.vector.tensor_tensor(
                    out=ltf, in0=ltf, in1=dk[:], op=Alu.bitwise_or
                )

            # ---- softmax phase 1: dist (Exp on ACT, both tiles) ----
            dist_t = []
            for t in range(NT):
                l_t = lt16[:, t, :].bitcast(dt.float32)   # [128, A]
                lm = work.tile([128, A], dt.float32, tag="lm")
                nc.vector.tensor_tensor(
                    out=lm[:], in0=l_t, in1=mterm[:, t, :], op=Alu.add
                )
                negm = work.tile([128, 1], dt.float32, tag="negm")
                nc.vector.tensor_reduce(
                    out=negm[:], in_=lm[:], axis=mybir.AxisListType.X,
                    op=Alu.max, negate=True,
                )
                et = work.tile([128, A], dt.float32, tag="et")
                zs = work.tile([128, 1], dt.float32, tag="zs")
                nc.scalar.activation(
                    out=et[:], in_=lm[:], func=ActF.Exp,
                    bias=negm[:], scale=1.0, accum_out=zs[:],
                )
                rz = work.tile([128, 1], dt.float32, tag="rz")
                nc.vector.reciprocal(out=rz[:], in_=zs[:])
                dt_ = work.tile([128, A], dt.float32, tag="dist")
                nc.vector.tensor_scalar(
                    out=dt_[:], in0=et[:],
                    scalar1=rz[:], scalar2=None, op0=Alu.mult,
                )
                dist_t.append(dt_)
                if t == 0:
                    nc.sync.dma_start(out=v_dist[:, t, :], in_=dt_[:])
                else:
                    nc.scalar.dma_start(out=v_dist[:, t, :], in_=dt_[:])

            # ---- softmax phase 2: entropy (Ln on ACT, both tiles) ----
            for t in range(NT):
                dt_ = dist_t[t]
                cl = work.tile([128, A], dt.float32, tag="cl")
                nc.vector.tensor_scalar(
                    out=cl[:], in0=dt_[:],
                    scalar1=1e-20, scalar2=None, op0=Alu.max,
                )
                lg = work.tile([128, A], dt.float32, tag="lg")
                nc.scalar.activation(out=lg[:], in_=cl[:], func=ActF.Ln)
                scr = work.tile([128, A], dt.float32, tag="scr")
                ent = work.tile([128, 1], dt.float32, tag="ent")
                nc.vector.tensor_tensor(
                    out=scr[:], in0=dt_[:], in1=lg[:], op=Alu.mult
                )
                nc.vector.tensor_reduce(
                    out=ent[:], in_=scr[:], axis=mybir.AxisListType.X,
                    op=Alu.add, negate=True,
                )
                nc.sync.dma_start(out=v_ent[:, t : t + 1], in_=ent[:])

    nc.compile()
    return nc


def _get_nc(rounds):
    key = ("nc", rounds)
    if key not in _CACHE:
        _CACHE[key] = _build(rounds)
    return _CACHE[key]


def _ensure_ntff_hook():
    """Install the antenv.axon_hooks shim so trace=True works under axon."""
    import sys
    import types

    try:
        from antenv.axon_hooks import get_axon_ntff_profile_hook  # noqa: F401
        return
    except ImportError:
        pass
    try:
        from trn_agent_boot.trn_boot import _ntff_profile_via_ctypes
        hook = _ntff_profile_via_ctypes("/opt/axon/libaxon_pjrt.so")
    except Exception:
        hook = None
    mod = types.ModuleType("antenv.axon_hooks")
    mod.get_axon_ntff_profile_hook = lambda: hook
    mod.set_axon_ntff_profile_hook = lambda h: None
    sys.modules["antenv.axon_hooks"] = mod
    import antenv

    antenv.axon_hooks = mod


def kernel(e, q, H, r_space, r_mask, entity_emb, rel_emb, W1, b1, W2, b2,
           trace=False, **run_kwargs):
    from concourse.bass_utils import run_bass_kernel_spmd

    if trace:
        _ensure_ntff_hook()

    e = np.asarray(e).astype(np.int32)
    q = np.asarray(q).astype(np.int32)
    H = np.ascontiguousarray(np.asarray(H, dtype=np.float32))
    r_space = np.asarray(r_space)
    r_mask = np.ascontiguousarray(np.asarray(r_mask, dtype=np.float32))
    entity_emb = np.ascontiguousarray(np.asarray(entity_emb, dtype=np.float32))
    rel_emb = np.ascontiguousarray(np.asarray(rel_emb, dtype=np.float32))
    rel_embT = np.ascontiguousarray(rel_emb.T)
    W1 = np.ascontiguousarray(np.asarray(W1, dtype=np.float32))
    W2 = np.ascontiguousarray(np.asarray(W2, dtype=np.float32))
    b1 = np.ascontiguousarray(np.asarray(b1, dtype=np.float32))
    b2 = np.ascontiguousarray(np.asarray(b2, dtype=np.float32))
    ident = np.eye(128, dtype=np.float32)

    inv0, jumps, M, R = _build_maps(r_space)
    inv16 = _pairs16(inv0)
    tilevec = ((np.arange(B) % BC) // 128)[:, None] * (2 * A)
    jmp16 = []
    for j in jumps:
        pj = _pairs16(j).astype(np.int32)
        pj = np.where(pj >= 0, pj + tilevec, -1).astype(np.int16)
        jmp16.append(pj)

    nc = _get_nc(R)
    in_maps = []
    for i in range(NCORES):
        sl = slice(i * BC, (i + 1) * BC)
        m = {
            "entity_emb": entity_emb,
            "rel_emb": rel_emb,
            "rel_embT": rel_embT,
            "W1": W1, "W2": W2, "b1": b1, "b2": b2,
            "e_idx": np.ascontiguousarray(e[sl]),
            "q_idx": np.ascontiguousarray(q[sl]),
            "inv16": np.ascontiguousarray(inv16[sl]),
            "r_mask": np.ascontiguousarray(r_mask[sl]),
            "H": np.ascontiguousarray(H[sl]),
            "ident": ident,
        }
        for k in range(R):
            m[f"jmp{k}"] = np.ascontiguousarray(jmp16[k][sl])
        in_maps.append(m)

    res = run_bass_kernel_spmd(
        nc, in_maps, core_ids=list(range(NCORES)), trace=trace, **run_kwargs
    )
    _CACHE["last_result"] = res

    dist = np.concatenate([r["dist"] for r in res.results], axis=0)
    entropy = np.concatenate([r["entropy"] for r in res.results], axis=0)
    return dist, entropy


# revision 11
# speedup vs baseline: 3.8957x; 1.0109x over previous
"""Trainium2 Bass kernel for the GraphSearchPolicy forward pass.

Math (see reference):
    E = entity_emb[e]; Q = rel_emb[q]; X = [E, H, Q]
    X1 = relu(X @ W1 + b1); X2 = X1 @ W2 + b2
    logit[b,a] = <rel_emb[r_space[b,a]], X2[b]> - (1-r_mask)*HUGE
    dist = softmax(logit); entropy = -sum(dist*log(clip(dist,1e-20)))

Strategy: data-parallel over B across 8 NeuronCores (256 rows each).

Key trick 1: instead of gathering 33MB of rel_emb rows per core, compute
P[b, r] = <X2[b], rel_emb[r]> for ALL r in [0,1000) with PE matmuls
(X2T stationary, rel_embT moving).

Key trick 2: the per-row gather logit[b,a] = P[b, r_space[b,a]] is
executed as a per-partition INVERSE SCATTER with gpsimd.local_scatter
(a fast streaming Q7 kernel with true per-partition indices):
  round 0:  dst[p, inv0[p,r]] = P[p,r]   (first occurrence of each r)
  round k:  dst[p, jump_{2^(k-1)}[p,a]] = cur[p,a], cur |= dst
where jump_j maps occurrence i of a value to occurrence i+j. After
ceil(log2(max_multiplicity)) rounds every duplicate slot is filled.
All maps are integer-only functions of r_space, precomputed host-side.
f32 values are moved bit-exactly as u16 pairs (local_scatter is 16-bit).
"""

import numpy as np

B, A = 2048, 256
NE, NR = 100000, 1000
DE, DR, DH = 128, 128, 256
ACTD = DE + DR            # 256, W1 output width
XDIM = DE + DH + DR       # 512
NCORES = 8
BC = B // NCORES          # 256 rows per core
NT = BC // 128            # 2 partition tiles per core
HUGE = float(np.float32(1e31))

_CACHE = {}


def _build_maps(r_space):
    """Integer-only index prep: inverse first-occurrence map plus
    log-doubling duplicate jump maps, all derived from r_space."""
    s = np.asarray(r_space).astype(np.int64)
    Bn, An = s.shape
    order = np.argsort(s, axis=1, kind="stable")
    s_sorted = np.take_along_axis(s, order, axis=1)
    new_grp = np.ones_like(s_sorted, dtype=bool)
    new_grp[:, 1:] = s_sorted[:, 1:] != s_sorted[:, :-1]
    idx_all = np.broadcast_to(np.arange(An)[None, :], (Bn, An))
    grp_start = np.where(new_grp, idx_all, 0)
    grp_start = np.maximum.accumulate(grp_start, axis=1)
    occ_sorted = idx_all - grp_start
    occ = np.empty_like(occ_sorted)
    np.put_along_axis(occ, order, occ_sorted, axis=1)
    M = int(occ.max()) + 1

    inv0 = np.full((Bn, NR), -1, np.int64)
    rows, cols = np.nonzero(occ == 0)
    inv0[rows, s[rows, cols]] = cols

    R = 0 if M <= 1 else int(np.ceil(np.log2(M)))
    jumps = []
    j = 1
    for _ in range(R):
        nxt_sorted = np.full((Bn, An), -1, np.int64)
        same = s_sorted[:, j:] == s_sorted[:, :-j]
        tmp = nxt_sorted[:, :-j]
        tmp[same] = order[:, j:][same]
        nxt_sorted[:, :-j] = tmp
        nxt = np.empty_like(nxt_sorted)
        np.put_along_axis(nxt, order, nxt_sorted, axis=1)
        jumps.append(nxt)
        j *= 2
    return inv0, jumps, M, R


def _pairs16(m):
    """Expand an f32-slot index map to its u16-pair index map (-1 kept)."""
    out = np.full((m.shape[0], 2 * m.shape[1]), -1, np.int16)
    valid = m >= 0
    out[:, 0::2] = np.where(valid, 2 * m, -1).astype(np.int16)
    out[:, 1::2] = np.where(valid, 2 * m + 1, -1).astype(np.int16)
    return out


def _build(rounds):
    import concourse.bass as bass
    import concourse.tile as tile
    from concourse import bacc, mybir

    dt = mybir.dt
    Alu = mybir.AluOpType
    ActF = mybir.ActivationFunctionType

    nc = bacc.Bacc(
        "TRN2", target_bir_lowering=False, debug=False, num_devices=NCORES
    )

    d_ee = nc.dram_tensor("entity_emb", [NE, DE], dt.float32, kind="ExternalInput")
    d_re = nc.dram_tensor("rel_emb", [NR, DR], dt.float32, kind="ExternalInput")
    d_reT = nc.dram_tensor("rel_embT", [DR, NR], dt.float32, kind="ExternalInput")
    d_w1 = nc.dram_tensor("W1", [XDIM, ACTD], dt.float32, kind="ExternalInput")
    d_w2 = nc.dram_tensor("W2", [ACTD, DR], dt.float32, kind="ExternalInput")
    d_b1 = nc.dram_tensor("b1", [ACTD], dt.float32, kind="ExternalInput")
    d_b2 = nc.dram_tensor("b2", [DR], dt.float32, kind="ExternalInput")
    d_e = nc.dram_tensor("e_idx", [BC], dt.int32, kind="ExternalInput")
    d_q = nc.dram_tensor("q_idx", [BC], dt.int32, kind="ExternalInput")
    d_inv = nc.dram_tensor("inv16", [BC, 2 * NR], dt.int16, kind="ExternalInput")
    d_jmp = [
        nc.dram_tensor(f"jmp{k}", [BC, 2 * A], dt.int16, kind="ExternalInput")
        for k in range(rounds)
    ]
    d_rm = nc.dram_tensor("r_mask", [BC, A], dt.float32, kind="ExternalInput")
    d_h = nc.dram_tensor("H", [BC, DH], dt.float32, kind="ExternalInput")
    d_id = nc.dram_tensor("ident", [128, 128], dt.float32, kind="ExternalInput")
    d_dist = nc.dram_tensor("dist", [BC, A], dt.float32, kind="ExternalOutput")
    d_ent = nc.dram_tensor("entropy", [BC], dt.float32, kind="ExternalOutput")

    # partition-tiled DRAM views: row (t*128 + p) -> [p, t, ...]
    v_e = d_e.ap().rearrange("(t p) -> p t", p=128)
    v_q = d_q.ap().rearrange("(t p) -> p t", p=128)
    v_inv = d_inv.ap().rearrange("(t p) c -> p t c", p=128)
    v_jmp = [d.ap().rearrange("(t p) c -> p t c", p=128) for d in d_jmp]
    v_rm = d_rm.ap().rearrange("(t p) a -> p t a", p=128)
    v_h = d_h.ap().rearrange("(t p) a -> p t a", p=128)
    v_w1 = d_w1.ap().rearrange("(t k) m -> k t m", k=128)   # [128, 4, 256]
    v_w2 = d_w2.ap().rearrange("(t k) m -> k t m", k=128)   # [128, 2, 128]
    v_b1 = d_b1.ap().rearrange("(t p) -> p t", p=128)       # [128, 2]
    v_b2 = d_b2.ap().rearrange("(t p) -> p t", p=128)       # [128, 1]
    v_reT = d_reT.ap().rearrange("(t p) r -> p t r", p=128)  # [128, 1, 1000]
    v_dist = d_dist.ap().rearrange("(t p) a -> p t a", p=128)
    v_ent = d_ent.ap().rearrange("(t p) -> p t", p=128)

    with tile.TileContext(nc) as tc:
        with (
            tc.tile_pool(name="const", bufs=1) as const,
            tc.tile_pool(name="work", bufs=2) as work,
            tc.tile_pool(name="psum", bufs=2, space="PSUM") as psum,
        ):
            # ---------------- input loads (spread across HWDGE queues) ---
            ident = const.tile([128, 128], dt.float32)
            nc.sync.dma_start(out=ident[:], in_=d_id.ap())
            h_sb = const.tile([128, NT, DH], dt.float32)
            nc.sync.dma_start(out=h_sb[:], in_=v_h)
            w1s = const.tile([128, 4, ACTD], dt.float32)
            nc.sync.dma_start(out=w1s[:], in_=v_w1)
            relT = const.tile([128, NR], dt.float32)
            nc.sync.dma_start(out=relT[:], in_=v_reT[:, 0, :])
            w2s = const.tile([128, 2, DR], dt.float32)
            nc.sync.dma_start(out=w2s[:], in_=v_w2)

            e_sb = const.tile([128, NT], dt.int32)
            nc.scalar.dma_start(out=e_sb[:], in_=v_e)
            q_sb = const.tile([128, NT], dt.int32)
            nc.scalar.dma_start(out=q_sb[:], in_=v_q)
            b1s = const.tile([128, 2], dt.float32)
            nc.scalar.dma_start(out=b1s[:], in_=v_b1)
            b2s = const.tile([128, 1], dt.float32)
            nc.scalar.dma_start(out=b2s[:], in_=v_b2)
            rm_sb = const.tile([128, NT, A], dt.float32)
            nc.scalar.dma_start(out=rm_sb[:], in_=v_rm)
            inv_sb = const.tile([128, NT, 2 * NR], dt.int16)
            nc.scalar.dma_start(out=inv_sb[:], in_=v_inv)
            jmp_sb = []
            for k in range(rounds):
                jt = const.tile([128, NT, 2 * A], dt.int16, tag=f"jmp{k}")
                nc.sync.dma_start(out=jt[:], in_=v_jmp[k])
                jmp_sb.append(jt)

            # ---------------- embedding gathers (Pool / SWDGE) ----------
            Eraw = work.tile([128, NT, DE], dt.float32, tag="Eraw")
            Qraw = work.tile([128, NT, DR], dt.float32, tag="Qraw")
            for t in range(NT):
                nc.gpsimd.indirect_dma_start(
                    out=Eraw[:, t, :],
                    out_offset=None,
                    in_=d_ee.ap(),
                    in_offset=bass.IndirectOffsetOnAxis(ap=e_sb[:, t : t + 1], axis=0),
                )
                nc.gpsimd.indirect_dma_start(
                    out=Qraw[:, t, :],
                    out_offset=None,
                    in_=d_re.ap(),
                    in_offset=bass.IndirectOffsetOnAxis(ap=q_sb[:, t : t + 1], axis=0),
                )

            # ---------------- build X^T k-tiles via PE transpose --------
            # XT[:, kt, :]: kt0 = E^T, kt1/kt2 = H^T halves, kt3 = Q^T
            XT = const.tile([128, 4, BC], dt.float32)
            X1T = const.tile([128, 2, BC], dt.float32)
            X2T = const.tile([128, BC], dt.float32)
            P16 = const.tile([128, NT, 2 * NR], dt.int16)
            lt16 = const.tile([128, NT, 2 * A], dt.int16)
            ltf = lt16[:].rearrange("p t c -> p (t c)")          # [128, 1024]
            mterm = const.tile([128, NT, A], dt.float32)

            for t in range(NT):
                # mask term only needs r_mask; hoist off the tail
                nc.vector.tensor_scalar(
                    out=mterm[:, t, :], in0=rm_sb[:, t, :],
                    scalar1=HUGE, scalar2=-HUGE,
                    op0=Alu.mult, op1=Alu.add,
                )
                bsl = slice(t * 128, t * 128 + 128)
                pE = psum.tile([128, 128], dt.float32, tag="ps_tr")
                nc.tensor.transpose(out=pE[:], in_=Eraw[:, t, :], identity=ident[:])
                nc.vector.tensor_copy(out=XT[:, 0, bsl], in_=pE[:])
                for u in range(2):
                    pH = psum.tile([128, 128], dt.float32, tag="ps_tr")
                    nc.tensor.transpose(
                        out=pH[:],
                        in_=h_sb[:, t, u * 128 : u * 128 + 128],
                        identity=ident[:],
                    )
                    nc.vector.tensor_copy(out=XT[:, 1 + u, bsl], in_=pH[:])
                pQ = psum.tile([128, 128], dt.float32, tag="ps_tr")
                nc.tensor.transpose(out=pQ[:], in_=Qraw[:, t, :], identity=ident[:])
                nc.vector.tensor_copy(out=XT[:, 3, bsl], in_=pQ[:])

                # ---- per-tile MLP so the first scatter starts early ----
                for mt in range(2):
                    ps = psum.tile([128, 128], dt.float32, tag="ps_mm")
                    kt_order = (1, 2, 0, 3)
                    for i, kt in enumerate(kt_order):
                        nc.tensor.matmul(
                            ps[:],
                            w1s[:, kt, mt * 128 : mt * 128 + 128],
                            XT[:, kt, bsl],
                            start=(i == 0),
                            stop=(i == 3),
                        )
                    nc.scalar.activation(
                        out=X1T[:, mt, bsl],
                        in_=ps[:],
                        func=ActF.Relu,
                        bias=b1s[:, mt : mt + 1],
                        scale=1.0,
                    )
                ps2 = psum.tile([128, 128], dt.float32, tag="ps_mm")
                for kt in range(2):
                    nc.tensor.matmul(
                        ps2[:],
                        w2s[:, kt, :],
                        X1T[:, kt, bsl],
                        start=(kt == 0),
                        stop=(kt == 1),
                    )
                nc.vector.tensor_scalar(
                    out=X2T[:, bsl], in0=ps2[:],
                    scalar1=b2s[:, 0:1], scalar2=None, op0=Alu.add,
                )

                # ---- P[b, r] = X2 @ rel_emb^T, written as u16 pairs ----
                for n0 in (0, 500):
                    pp = psum.tile([128, 500], dt.float32, tag="ps_p")
                    nc.tensor.matmul(
                        pp[:],
                        X2T[:, bsl],
                        relT[:, n0 : n0 + 500],
                        start=True,
                        stop=True,
                    )
                    nc.vector.tensor_copy(
                        out=P16[:, t, 2 * n0 : 2 * n0 + 1000].bitcast(dt.float32),
                        in_=pp[:],
                    )

                # ---- scatter round 0 (first occurrences) ----
                nc.gpsimd.local_scatter(
                    lt16[:, t, :], P16[:, t, :], inv_sb[:, t, :],
                    channels=128, num_elems=2 * A, num_idxs=2 * NR,
                )

            # ---- duplicate-fill jump rounds, both tiles per scatter ----
            for k in range(rounds):
                dk = work.tile([128, NT * 2 * A], dt.int16, tag="dk")
                nc.gpsimd.local_scatter(
                    dk[:], ltf, jmp_sb[k][:].rearrange("p t c -> p (t c)"),
                    channels=128, num_elems=NT * 2 * A, num_idxs=NT * 2 * A,
                )
                nc.vector.tensor_tensor(
                    out=ltf, in0=ltf, in1=dk[:], op=Alu.bitwise_or
                )

            # ---- softmax phase 1: dist (Exp on ACT, both tiles) ----
            dist_t = []
            for t in range(NT):
                l_t = lt16[:, t, :].bitcast(dt.float32)   # [128, A]
                lm = work.tile([128, A], dt.float32, tag="lm")
                nc.vector.tensor_tensor(
                    out=lm[:], in0=l_t, in1=mterm[:, t, :], op=Alu.add
                )
                negm = work.tile([128, 1], dt.float32, tag="negm")
                nc.vector.tensor_reduce(
                    out=negm[:], in_=lm[:], axis=mybir.AxisListType.X,
                    op=Alu.max, negate=True,
                )
                et = work.tile([128, A], dt.float32, tag="et")
                zs = work.tile([128, 1], dt.float32, tag="zs")
                nc.scalar.activation(
                    out=et[:], in_=lm[:], func=ActF.Exp,
                    bias=negm[:], scale=1.0, accum_out=zs[:],
                )
                rz = work.tile([128, 1], dt.float32, tag="rz")
                nc.vector.reciprocal(out=rz[:], in_=zs[:])
                dt_ = work.tile([128, A], dt.float32, tag="dist")
                nc.vector.tensor_scalar(
                    out=dt_[:], in0=et[:],
                    scalar1=rz[:], scalar2=None, op0=Alu.mult,
                )
                dist_t.append(dt_)
                if t == 0:
                    nc.sync.dma_start(out=v_dist[:, t, :], in_=dt_[:])
                else:
                    nc.scalar.dma_start(out=v_dist[:, t, :], in_=dt_[:])

            # ---- softmax phase 2: entropy (Ln on ACT, both tiles) ----
            for t in range(NT):
                dt_ = dist_t[t]
                cl = work.tile([128, A], dt.float32, tag="cl")
                nc.vector.tensor_scalar(
                    out=cl[:], in0=dt_[:],
                    scalar1=1e-20, scalar2=None, op0=Alu.max,
                )
                lg = work.tile([128, A], dt.float32, tag="lg")
                nc.scalar.activation(out=lg[:], in_=cl[:], func=ActF.Ln)
                scr = work.tile([128, A], dt.float32, tag="scr")
                ent = work.tile([128, 1], dt.float32, tag="ent")
                nc.vector.tensor_tensor(
                    out=scr[:], in0=dt_[:], in1=lg[:], op=Alu.mult
                )
                nc.vector.tensor_reduce(
                    out=ent[:], in_=scr[:], axis=mybir.AxisListType.X,
                    op=Alu.add, negate=True,
                )
                nc.sync.dma_start(out=v_ent[:, t : t + 1], in_=ent[:])

    nc.compile()
    return nc


def _get_nc(rounds):
    key = ("nc", rounds)
    if key not in _CACHE:
        _CACHE[key] = _build(rounds)
    return _CACHE[key]


def _ensure_ntff_hook():
    """Install the antenv.axon_hooks shim so trace=True works under axon."""
    import sys
    import types

    try:
        from antenv.axon_hooks import get_axon_ntff_profile_hook  # noqa: F401
        return
    except ImportError:
        pass
    try:
        from trn_agent_boot.trn_boot import _ntff_profile_via_ctypes
        hook = _ntff_profile_via_ctypes("/opt/axon/libaxon_pjrt.so")
    except Exception:
        hook = None
    mod = types.ModuleType("antenv.axon_hooks")
    mod.get_axon_ntff_profile_hook = lambda: hook
    mod.set_axon_ntff_profile_hook = lambda h: None
    sys.modules["antenv.axon_hooks"] = mod
    import antenv

    antenv.axon_hooks = mod


def kernel(e, q, H, r_space, r_mask, entity_emb, rel_emb, W1, b1, W2, b2,
           trace=False, **run_kwargs):
    from concourse.bass_utils import run_bass_kernel_spmd

    if trace:
        _ensure_ntff_hook()

    e = np.asarray(e).astype(np.int32)
    q = np.asarray(q).astype(np.int32)
    H = np.ascontiguousarray(np.asarray(H, dtype=np.float32))
    r_space = np.asarray(r_space)
    r_mask = np.ascontiguousarray(np.asarray(r_mask, dtype=np.float32))
    entity_emb = np.ascontiguousarray(np.asarray(entity_emb, dtype=np.float32))
    rel_emb = np.ascontiguousarray(np.asarray(rel_emb, dtype=np.float32))
    rel_embT = np.ascontiguousarray(rel_emb.T)
    W1 = np.ascontiguousarray(np.asarray(W1, dtype=np.float32))
    W2 = np.ascontiguousarray(np.asarray(W2, dtype=np.float32))
    b1 = np.ascontiguousarray(np.asarray(b1, dtype=np.float32))
    b2 = np.ascontiguousarray(np.asarray(b2, dtype=np.float32))
    ident = np.eye(128, dtype=np.float32)

    inv0, jumps, M, R = _build_maps(r_space)
    inv16 = _pairs16(inv0)
    tilevec = ((np.arange(B) % BC) // 128)[:, None] * (2 * A)
    jmp16 = []
    for j in jumps:
        pj = _pairs16(j).astype(np.int32)
        pj = np.where(pj >= 0, pj + tilevec, -1).astype(np.int16)
        jmp16.append(pj)

    nc = _get_nc(R)
    in_maps = []
    for i in range(NCORES):
        sl = slice(i * BC, (i + 1) * BC)
        m = {
            "entity_emb": entity_emb,
            "rel_emb": rel_emb,
            "rel_embT": rel_embT,
            "W1": W1, "W2": W2, "b1": b1, "b2": b2,
            "e_idx": np.ascontiguousarray(e[sl]),
            "q_idx": np.ascontiguousarray(q[sl]),
            "inv16": np.ascontiguousarray(inv16[sl]),
            "r_mask": np.ascontiguousarray(r_mask[sl]),
            "H": np.ascontiguousarray(H[sl]),
            "ident": ident,
        }
        for k in range(R):
            m[f"jmp{k}"] = np.ascontiguousarray(jmp16[k][sl])
        in_maps.append(m)

    res = run_bass_kernel_spmd(
        nc, in_maps, core_ids=list(range(NCORES)), trace=trace, **run_kwargs
    )
    _CACHE["last_result"] = res

    dist = np.concatenate([r["dist"] for r in res.results], axis=0)
    entropy = np.concatenate([r["entropy"] for r in res.results], axis=0)
    return dist, entropy
